# revision 11
# baseline (speedup 1.0000x reference)
"""Trainium2 Bass kernel for nn_AttackHead (GNN edge-scorer attack head).

Strategy (8 NeuronCores, data-parallel over edges):
- Host: cast node_embeddings to bf16, shard 500k edges into 8x62500,
  bucket each core's edges by (src%4, tgt%4) so dma_gather's int16 index
  window (25000 super-rows of 4 nodes, 2048B stride) covers the table.
- Device per core: dma_gather(transpose=True) fetches gathered embeddings
  directly in [feature, edge] layout; layer-1 MLPs run as bf16 matmuls
  producing H^T/Ha^T; layer-2 + biases + mask penalties fold into matmuls
  against a host-built one-hot of (max_sendable, bad_edge, self_loop), so
  the whole edge pipeline is PE+ACT only (no DVE, which would lock GpSimd
  out of SBUF and stall descriptor generation).
- Host: inverse-permute device outputs back to original edge order.
"""
import numpy as np
import ml_dtypes

N_NODES = 100000
N_EDGES = 500000
D = 256          # embed dim
HID = 256        # edge scorer hidden
AH = 128         # army scorer hidden
MA = 64          # MAX_ARMY
NC = 8           # cores
ES = N_EDGES // NC  # edges per core
NSUPER = N_NODES // 4  # super-rows of 4 nodes
GCALL = 896      # gather granule (single_packet tx limit is ~992)
NEG = -1e9

bf16 = ml_dtypes.bfloat16

LAST = {}  # stash of the last run's BassKernelResults (for test harnesses)


def _calls_for(pb):
    """Split a bucket of pb edges (multiple of 128) into gather calls."""
    calls = []
    while pb >= GCALL + 128:
        calls.append(GCALL)
        pb -= GCALL
    if pb:
        calls.append(pb)
    return calls


def _subtiles(n):
    """Split a gather call of n edges into compute tiles (<=512, mult of 128)."""
    tiles = []
    while n > 512:
        tiles.append(512)
        n -= 512
    if n:
        tiles.append(n)
    return tiles


def _build(bucket_sizes):
    """Build and compile the SPMD Bass program for the given per-bucket
    padded sizes (shared across cores). Returns (nc, tpad)."""
    import concourse.bass as bass
    import concourse.bacc as bacc
    import concourse.tile as tile
    from concourse.tile import add_dep_helper
    from concourse import mybir

    tpad = sum(bucket_sizes)
    nc = bacc.Bacc("TRN2", target_bir_lowering=False, debug=False,
                   num_devices=NC, num_swdge_queues=4)

    table_t = nc.dram_tensor("table", [N_NODES, D], mybir.dt.bfloat16,
                             kind="ExternalInput")
    sidx_t = nc.dram_tensor("sidx", [128, tpad // 16], mybir.dt.int16,
                            kind="ExternalInput")
    tidx_t = nc.dram_tensor("tidx", [128, tpad // 16], mybir.dt.int16,
                            kind="ExternalInput")
    oh_t = nc.dram_tensor("oh", [120, tpad], mybir.dt.bfloat16,
                          kind="ExternalInput")
    w1_t = nc.dram_tensor("w1", [128, 8, 128], mybir.dt.bfloat16,
                          kind="ExternalInput")
    wa1_t = nc.dram_tensor("wa1", [128, 4, 128], mybir.dt.bfloat16,
                           kind="ExternalInput")
    w2x_t = nc.dram_tensor("w2x", [128, 4, 65], mybir.dt.bfloat16,
                           kind="ExternalInput")
    b1_t = nc.dram_tensor("b1c", [128, 3], mybir.dt.float32,
                          kind="ExternalInput")
    out_t = nc.dram_tensor("out", [65, tpad], mybir.dt.float32,
                           kind="ExternalOutput")

    with tile.TileContext(nc) as tc:
        with (
            tc.tile_pool(name="const", bufs=1) as cpool,
            tc.tile_pool(name="x", bufs=3) as xpool,
            tc.tile_pool(name="h", bufs=2) as hpool,
            tc.tile_pool(name="oh", bufs=2) as ohpool,
            tc.tile_pool(name="o", bufs=3) as opool,
            tc.tile_pool(name="ph", bufs=2, space="PSUM") as phpool,
            tc.tile_pool(name="po", bufs=2, space="PSUM") as popool,
        ):
            w1_sb = cpool.tile([128, 8, 128], mybir.dt.bfloat16)
            nc.sync.dma_start(out=w1_sb[:], in_=w1_t.ap())
            wa1_sb = cpool.tile([128, 4, 128], mybir.dt.bfloat16)
            nc.sync.dma_start(out=wa1_sb[:], in_=wa1_t.ap())
            w2x_sb = cpool.tile([128, 4, 65], mybir.dt.bfloat16)
            nc.sync.dma_start(out=w2x_sb[:], in_=w2x_t.ap())
            b1_sb = cpool.tile([128, 3], mybir.dt.float32)
            nc.sync.dma_start(out=b1_sb[:], in_=b1_t.ap())
            sidx_sb = cpool.tile([128, tpad // 16], mybir.dt.int16)
            nc.sync.dma_start(out=sidx_sb[:], in_=sidx_t.ap())
            tidx_sb = cpool.tile([128, tpad // 16], mybir.dt.int16)
            nc.sync.dma_start(out=tidx_sb[:], in_=tidx_t.ap())

            relu = mybir.ActivationFunctionType.Relu
            copyf = mybir.ActivationFunctionType.Copy

            d0 = 0      # global device-edge offset
            qn = 0      # swdge queue rotation
            # Gathers must stay in program order: Tile assigns SWDGE DMAs
            # to 8 completion-sem lanes round-robin in *scheduled* order,
            # and per-lane sem counting is only sound when each lane holds
            # a single in-order queue. Chain them so lane k <-> queue k%4.
            prev_gather = None
            for b, pb in enumerate(bucket_sizes):
                if pb == 0:
                    continue
                rs, rt = b >> 2, b & 3
                src_ap = bass.AP(tensor=table_t, offset=rs * D,
                                 ap=[[1024, NSUPER], [1, D]])
                tgt_ap = bass.AP(tensor=table_t, offset=rt * D,
                                 ap=[[1024, NSUPER], [1, D]])
                for n in _calls_for(pb):
                    o16 = d0 // 16
                    xs = xpool.tile([128, 2, n], mybir.dt.bfloat16, tag="xs")
                    g1 = nc.gpsimd.dma_gather(
                        out_ap=xs[:], in_ap=src_ap,
                        idxs_ap=sidx_sb[:, o16:o16 + n // 16],
                        num_idxs=n, num_idxs_reg=n, elem_size=D,
                        elem_step=1024, transpose=True,
                        queue_num=qn)
                    qn = (qn + 1) % 4
                    xt = xpool.tile([128, 2, n], mybir.dt.bfloat16, tag="xt")
                    g2 = nc.gpsimd.dma_gather(
                        out_ap=xt[:], in_ap=tgt_ap,
                        idxs_ap=tidx_sb[:, o16:o16 + n // 16],
                        num_idxs=n, num_idxs_reg=n, elem_size=D,
                        elem_step=1024, transpose=True,
                        queue_num=qn)
                    qn = (qn + 1) % 4
                    for g in (g1, g2):
                        if prev_gather is not None:
                            add_dep_helper(g.ins, prev_gather.ins, sync=False,
                                           reason="gather issue order")
                        prev_gather = g

                    toff = 0  # offset within this gather call
                    for w in _subtiles(n):
                        sl = slice(toff, toff + w)
                        # ---- layer 1: H^T chunks + Ha^T, K=512 over 4 chunks
                        ph = [phpool.tile([128, 512], mybir.dt.float32,
                                          tag=f"ph{m}", name=f"ph{m}")
                              for m in range(3)]
                        for m in range(3):
                            for k in range(4):
                                lhsT = (w1_sb[:, k * 2 + m, :] if m < 2
                                        else wa1_sb[:, k, :])
                                rhs = (xs if k < 2 else xt)[:, k % 2, sl]
                                nc.tensor.matmul(
                                    out=ph[m][:, :w], lhsT=lhsT, rhs=rhs,
                                    start=(k == 0), stop=(k == 3))
                        hts = []
                        for m in range(3):
                            ht = hpool.tile([128, 512], mybir.dt.bfloat16,
                                            tag=f"ht{m}")
                            nc.scalar.activation(
                                out=ht[:, :w], in_=ph[m][:, :w], func=relu,
                                bias=b1_sb[:, m:m + 1])
                            hts.append(ht)

                        # ---- one-hot combo slice for this tile
                        oh_sb = ohpool.tile([120, 512], mybir.dt.bfloat16,
                                            tag="oh")
                        nc.sync.dma_start(out=oh_sb[:, :w],
                                          in_=oh_t.ap()[:, d0 + toff:d0 + toff + w])

                        # ---- layer 2 (+bias+mask+penalty): out [65, w], the
                        # four stationaries (W2ext x3, ComboM) are constant
                        # across tiles so LDWEIGHTS hides under N=512 streams
                        po = popool.tile([65, 512], mybir.dt.float32,
                                         tag="po")
                        for i in range(3):
                            nc.tensor.matmul(out=po[:, :w],
                                             lhsT=w2x_sb[:, i, :],
                                             rhs=hts[i][:, :w],
                                             start=(i == 0), stop=False)
                        nc.tensor.matmul(out=po[:, :w],
                                         lhsT=w2x_sb[:120, 3, :],
                                         rhs=oh_sb[:, :w],
                                         start=False, stop=True)
                        ot = opool.tile([65, 512], mybir.dt.float32, tag="ot")
                        nc.scalar.activation(out=ot[:, :w], in_=po[:, :w],
                                             func=copyf)
                        nc.sync.dma_start(
                            out=out_t.ap()[:, d0 + toff:d0 + toff + w],
                            in_=ot[:, :w])
                        toff += w
                    d0 += n

    nc.compile()
    return nc, tpad


def kernel(node_embeddings, action_edges, army_counts,
           W1, b1, W2, b2, Wa1, ba1, Wa2, ba2):
    from concourse.bass_utils import run_bass_kernel_spmd

    node_embeddings = np.asarray(node_embeddings)
    action_edges = np.asarray(action_edges)
    army_counts = np.asarray(army_counts)
    W1 = np.asarray(W1, dtype=np.float32)
    b1 = np.asarray(b1, dtype=np.float32)
    W2 = np.asarray(W2, dtype=np.float32)
    b2 = np.asarray(b2, dtype=np.float32)
    Wa1 = np.asarray(Wa1, dtype=np.float32)
    ba1 = np.asarray(ba1, dtype=np.float32)
    Wa2 = np.asarray(Wa2, dtype=np.float32)
    ba2 = np.asarray(ba2, dtype=np.float32)

    table = node_embeddings.astype(bf16)

    src = action_edges[:, 0].astype(np.int64)
    tgt = action_edges[:, 1].astype(np.int64)

    # ---- per-core bucketing by (src%4, tgt%4)
    cores = []
    for c in range(NC):
        s = src[c * ES:(c + 1) * ES]
        t = tgt[c * ES:(c + 1) * ES]
        key = (s & 3) * 4 + (t & 3)
        order = np.argsort(key, kind="stable")
        counts = np.bincount(key, minlength=16)
        cores.append((s, t, order, counts))

    max_counts = np.max([c[3] for c in cores], axis=0)
    bucket_sizes = [int(-(-m // 128) * 128) for m in max_counts]
    tpad = sum(bucket_sizes)

    nc, tpad2 = _build(bucket_sizes)
    assert tpad2 == tpad

    # ---- combo metadata matrix [120, 65]
    combo_m = np.zeros((128, 4, 65), dtype=np.float32)
    iota = np.arange(MA)
    for k in range(30):
        for bad in range(2):
            for sf in range(2):
                cidx = k * 4 + bad * 2 + sf
                row = np.where(iota <= k - 1, ba2, NEG)
                combo_m[cidx, 3, :MA] = row
                combo_m[cidx, 3, 64] = b2[0] - 1.0 * bad - 100.0 * sf
    # layer-2 weights, paired with lhsT = hts[i] in the device loop:
    # [0] H0 -> edge col, [1] H1 -> edge col, [2] Ha -> army cols
    combo_m[:128, 0, 64] = W2[:128, 0]
    combo_m[:128, 1, 64] = W2[128:, 0]
    combo_m[:, 2, :MA] = Wa2  # [128, 64]

    w1_in = np.zeros((128, 8, 128), dtype=np.float32)
    for k in range(4):
        for m in range(2):
            w1_in[:, k * 2 + m, :] = W1[k * 128:(k + 1) * 128,
                                        m * 128:(m + 1) * 128]
    wa1_in = np.zeros((128, 4, 128), dtype=np.float32)
    for k in range(4):
        wa1_in[:, k, :] = Wa1[k * 128:(k + 1) * 128, :]
    b1_in = np.zeros((128, 3), dtype=np.float32)
    b1_in[:, 0] = b1[:128]
    b1_in[:, 1] = b1[128:]
    b1_in[:, 2] = ba1

    # ---- per-core device inputs
    boffs = np.concatenate([[0], np.cumsum(bucket_sizes)]).astype(np.int64)
    call_layout = []  # (start, n) per call, shared across cores
    for b, pb in enumerate(bucket_sizes):
        off = int(boffs[b])
        for n in _calls_for(pb):
            call_layout.append((off, n))
            off += n

    in_maps = []
    perms = []  # per-core: device position -> original local edge idx (-1 pad)
    for c in range(NC):
        s, t, order, counts = cores[c]
        sl = (s >> 2).astype(np.int16)
        tl = (t >> 2).astype(np.int16)
        sa = army_counts[s].astype(np.int64)
        ta = army_counts[t].astype(np.int64)
        k_ms = np.clip(sa - 1, -1, 28) + 1
        bad = ((sa <= 2) | (ta >= 3 * sa)).astype(np.int64)
        sf = (s == t).astype(np.int64)
        combo = (k_ms * 4 + bad * 2 + sf).astype(np.int64)

        sidx = np.zeros(tpad, dtype=np.int16)
        tidx = np.zeros(tpad, dtype=np.int16)
        ohcol = np.full(tpad, -1, dtype=np.int64)
        perm = np.full(tpad, -1, dtype=np.int64)
        cum = np.concatenate([[0], np.cumsum(counts)]).astype(np.int64)
        for b in range(16):
            ids = order[cum[b]:cum[b + 1]]
            off = int(boffs[b])
            perm[off:off + len(ids)] = ids
            sidx[off:off + len(ids)] = sl[ids]
            tidx[off:off + len(ids)] = tl[ids]
            ohcol[off:off + len(ids)] = combo[ids]

        # wrap indices per gather call: [128, tpad//16]
        sw = np.zeros((128, tpad // 16), dtype=np.int16)
        tw = np.zeros((128, tpad // 16), dtype=np.int16)
        for off, n in call_layout:
            blk_s = sidx[off:off + n].reshape(n // 16, 16).T
            blk_t = tidx[off:off + n].reshape(n // 16, 16).T
            sw[:, off // 16:(off + n) // 16] = np.tile(blk_s, (8, 1))
            tw[:, off // 16:(off + n) // 16] = np.tile(blk_t, (8, 1))

        oh = np.zeros((120, tpad), dtype=bf16)
        valid = ohcol >= 0
        oh[ohcol[valid], np.nonzero(valid)[0]] = bf16(1.0)

        in_maps.append({
            "table": table,
            "sidx": sw,
            "tidx": tw,
            "oh": oh,
            "w1": w1_in.astype(bf16),
            "wa1": wa1_in.astype(bf16),
            "w2x": combo_m.astype(bf16),
            "b1c": b1_in,
        })
        perms.append(perm)

    res = run_bass_kernel_spmd(nc, in_maps, core_ids=list(range(NC)))
    LAST["res"] = res

    edge_logits = np.zeros(N_EDGES, dtype=np.float32)
    army_logits = np.zeros((N_EDGES, MA), dtype=np.float32)
    for c in range(NC):
        dev = res.results[c]["out"].T  # [65, tpad] -> [tpad, 65]
        perm = perms[c]
        valid = perm >= 0
        gidx = c * ES + perm[valid]
        edge_logits[gidx] = dev[valid, 64]
        army_logits[gidx] = dev[valid, :MA]
    return edge_logits, army_logits


# revision 16
# speedup vs baseline: 1.2281x; 1.2281x over previous
"""Trainium2 Bass kernel for nn_AttackHead (GNN edge-scorer attack head).

Strategy (8 NeuronCores, data-parallel over edges):
- Host: cast node_embeddings to bf16, shard 500k edges into 8x62500,
  bucket each core's edges by (src%4, tgt%4) so dma_gather's int16 index
  window (25000 super-rows of 4 nodes, 2048B stride) covers the table.
- Device per core: dma_gather(transpose=True) fetches gathered embeddings
  directly in [feature, edge] layout; layer-1 MLPs run as bf16 matmuls
  producing H^T/Ha^T; layer-2 + biases + mask penalties fold into matmuls
  against a host-built one-hot of (max_sendable, bad_edge, self_loop), so
  the whole edge pipeline is PE+ACT only (no DVE, which would lock GpSimd
  out of SBUF and stall descriptor generation).
- Host: inverse-permute device outputs back to original edge order.
"""
import numpy as np
import ml_dtypes

N_NODES = 100000
N_EDGES = 500000
D = 256          # embed dim
HID = 256        # edge scorer hidden
AH = 128         # army scorer hidden
MA = 64          # MAX_ARMY
NC = 8           # cores
ES = N_EDGES // NC  # edges per core
NSUPER = N_NODES // 4  # super-rows of 4 nodes
GCALL = 896      # gather granule (single_packet tx desc limit; mult of 128)
NEG = -1e9

bf16 = ml_dtypes.bfloat16

LAST = {}  # stash of the last run's BassKernelResults (for test harnesses)


def _calls_for(pb):
    """Split a bucket of pb edges (multiple of 128) into gather calls."""
    calls = []
    while pb >= GCALL + 128:
        calls.append(GCALL)
        pb -= GCALL
    if pb:
        calls.append(pb)
    return calls


def _subtiles(n):
    """Split a gather call of n edges into compute tiles (<=512, mult of 128)."""
    tiles = []
    while n > 512:
        tiles.append(512)
        n -= 512
    if n:
        tiles.append(n)
    return tiles


def _build(bucket_sizes):
    """Build and compile the SPMD Bass program for the given per-bucket
    padded sizes (shared across cores). Returns (nc, tpad)."""
    import concourse.bass as bass
    import concourse.bacc as bacc
    import concourse.tile as tile
    from concourse.tile import add_dep_helper
    from concourse import mybir

    tpad = sum(bucket_sizes)
    nc = bacc.Bacc("TRN2", target_bir_lowering=False, debug=False,
                   num_devices=NC, num_swdge_queues=4)

    table_t = nc.dram_tensor("table", [N_NODES, D], mybir.dt.bfloat16,
                             kind="ExternalInput")
    sidx_t = nc.dram_tensor("sidx", [128, tpad // 16], mybir.dt.int16,
                            kind="ExternalInput")
    tidx_t = nc.dram_tensor("tidx", [128, tpad // 16], mybir.dt.int16,
                            kind="ExternalInput")
    oh_t = nc.dram_tensor("oh", [120, tpad], mybir.dt.bfloat16,
                          kind="ExternalInput")
    w1_t = nc.dram_tensor("w1", [128, 8, 128], mybir.dt.bfloat16,
                          kind="ExternalInput")
    wa1_t = nc.dram_tensor("wa1", [128, 4, 128], mybir.dt.bfloat16,
                           kind="ExternalInput")
    w2x_t = nc.dram_tensor("w2x", [128, 4, 65], mybir.dt.bfloat16,
                           kind="ExternalInput")
    b1_t = nc.dram_tensor("b1c", [128, 3], mybir.dt.float32,
                          kind="ExternalInput")
    out_t = nc.dram_tensor("out", [128, (tpad // 128) * 65], mybir.dt.float32,
                           kind="ExternalOutput")

    with tile.TileContext(nc) as tc:
        with (
            tc.tile_pool(name="const", bufs=1) as cpool,
            tc.tile_pool(name="x", bufs=4) as xpool,
            tc.tile_pool(name="h", bufs=2) as hpool,
            tc.tile_pool(name="oh", bufs=2) as ohpool,
            tc.tile_pool(name="o", bufs=3) as opool,
            tc.tile_pool(name="ph", bufs=2, space="PSUM") as phpool,
            tc.tile_pool(name="po", bufs=2, space="PSUM") as popool,
        ):
            w1_sb = cpool.tile([128, 8, 128], mybir.dt.bfloat16)
            nc.sync.dma_start(out=w1_sb[:], in_=w1_t.ap())
            wa1_sb = cpool.tile([128, 4, 128], mybir.dt.bfloat16)
            nc.sync.dma_start(out=wa1_sb[:], in_=wa1_t.ap())
            w2x_sb = cpool.tile([128, 4, 65], mybir.dt.bfloat16)
            nc.sync.dma_start(out=w2x_sb[:], in_=w2x_t.ap())
            b1_sb = cpool.tile([128, 3], mybir.dt.float32)
            nc.sync.dma_start(out=b1_sb[:], in_=b1_t.ap())
            sidx_sb = cpool.tile([128, tpad // 16], mybir.dt.int16)
            nc.sync.dma_start(out=sidx_sb[:], in_=sidx_t.ap())
            tidx_sb = cpool.tile([128, tpad // 16], mybir.dt.int16)
            nc.sync.dma_start(out=tidx_sb[:], in_=tidx_t.ap())

            relu = mybir.ActivationFunctionType.Relu
            copyf = mybir.ActivationFunctionType.Copy

            d0 = 0      # global device-edge offset
            qn = 0      # swdge queue rotation
            # Gathers must stay in program order: Tile assigns SWDGE DMAs
            # to 8 completion-sem lanes round-robin in *scheduled* order,
            # and per-lane sem counting is only sound when each lane holds
            # a single in-order queue. Chain them so lane k <-> queue k%4.
            prev_gather = None
            for b, pb in enumerate(bucket_sizes):
                if pb == 0:
                    continue
                rs, rt = b >> 2, b & 3
                src_ap = bass.AP(tensor=table_t, offset=rs * D,
                                 ap=[[1024, NSUPER], [1, D]])
                tgt_ap = bass.AP(tensor=table_t, offset=rt * D,
                                 ap=[[1024, NSUPER], [1, D]])
                for n in _calls_for(pb):
                    o16 = d0 // 16
                    xs = xpool.tile([128, 2, n], mybir.dt.bfloat16, tag="xs")
                    g1 = nc.gpsimd.dma_gather(
                        out_ap=xs[:], in_ap=src_ap,
                        idxs_ap=sidx_sb[:, o16:o16 + n // 16],
                        num_idxs=n, num_idxs_reg=n, elem_size=D,
                        elem_step=1024, transpose=True,
                        queue_num=qn)
                    qn = (qn + 1) % 4
                    xt = xpool.tile([128, 2, n], mybir.dt.bfloat16, tag="xt")
                    g2 = nc.gpsimd.dma_gather(
                        out_ap=xt[:], in_ap=tgt_ap,
                        idxs_ap=tidx_sb[:, o16:o16 + n // 16],
                        num_idxs=n, num_idxs_reg=n, elem_size=D,
                        elem_step=1024, transpose=True,
                        queue_num=qn)
                    qn = (qn + 1) % 4
                    for g in (g1, g2):
                        if prev_gather is not None:
                            add_dep_helper(g.ins, prev_gather.ins, sync=False,
                                           reason="gather issue order")
                        prev_gather = g

                    toff = 0  # offset within this gather call
                    for w in _subtiles(n):
                        sl = slice(toff, toff + w)
                        # ---- layer 1: H^T chunks + Ha^T, K=512 over 4 chunks
                        ph = [phpool.tile([128, 512], mybir.dt.float32,
                                          tag=f"ph{m}", name=f"ph{m}")
                              for m in range(3)]
                        for m in range(3):
                            for k in range(4):
                                lhsT = (w1_sb[:, k * 2 + m, :] if m < 2
                                        else wa1_sb[:, k, :])
                                rhs = (xs if k < 2 else xt)[:, k % 2, sl]
                                nc.tensor.matmul(
                                    out=ph[m][:, :w], lhsT=lhsT, rhs=rhs,
                                    start=(k == 0), stop=(k == 3))
                        hts = []
                        for m in range(3):
                            ht = hpool.tile([128, 512], mybir.dt.bfloat16,
                                            tag=f"ht{m}")
                            nc.scalar.activation(
                                out=ht[:, :w], in_=ph[m][:, :w], func=relu,
                                bias=b1_sb[:, m:m + 1])
                            hts.append(ht)

                        # ---- one-hot combo slice for this tile
                        oh_sb = ohpool.tile([120, 512], mybir.dt.bfloat16,
                                            tag="oh")
                        nc.sync.dma_start(out=oh_sb[:, :w],
                                          in_=oh_t.ap()[:, d0 + toff:d0 + toff + w])

                        # ---- layer 2 (+bias+mask+penalty), per 128-edge chunk
                        nec = w // 128
                        po = popool.tile([128, 4, 65], mybir.dt.float32,
                                         tag="po")
                        for ec in range(nec):
                            esl = slice(ec * 128, (ec + 1) * 128)
                            nc.tensor.matmul(out=po[:, ec, :],
                                             lhsT=hts[0][:, :w][:, esl],
                                             rhs=w2x_sb[:, 0, :],
                                             start=True, stop=False)
                            nc.tensor.matmul(out=po[:, ec, :],
                                             lhsT=hts[1][:, :w][:, esl],
                                             rhs=w2x_sb[:, 1, :],
                                             start=False, stop=False)
                            nc.tensor.matmul(out=po[:, ec, :],
                                             lhsT=hts[2][:, :w][:, esl],
                                             rhs=w2x_sb[:, 2, :],
                                             start=False, stop=False)
                            nc.tensor.matmul(out=po[:, ec, :],
                                             lhsT=oh_sb[:, :w][:, esl],
                                             rhs=w2x_sb[:120, 3, :],
                                             start=False, stop=True)
                        ot = opool.tile([128, 4, 65], mybir.dt.float32,
                                        tag="ot")
                        nc.scalar.activation(out=ot[:, :nec, :],
                                             in_=po[:, :nec, :], func=copyf)
                        oc = (d0 + toff) // 128 * 65
                        nc.sync.dma_start(
                            out=out_t.ap()[:, oc:oc + nec * 65],
                            in_=ot[:, :nec, :])
                        toff += w
                    d0 += n

    nc.compile()
    return nc, tpad


def kernel(node_embeddings, action_edges, army_counts,
           W1, b1, W2, b2, Wa1, ba1, Wa2, ba2):
    from concourse.bass_utils import run_bass_kernel_spmd

    node_embeddings = np.asarray(node_embeddings)
    action_edges = np.asarray(action_edges)
    army_counts = np.asarray(army_counts)
    W1 = np.asarray(W1, dtype=np.float32)
    b1 = np.asarray(b1, dtype=np.float32)
    W2 = np.asarray(W2, dtype=np.float32)
    b2 = np.asarray(b2, dtype=np.float32)
    Wa1 = np.asarray(Wa1, dtype=np.float32)
    ba1 = np.asarray(ba1, dtype=np.float32)
    Wa2 = np.asarray(Wa2, dtype=np.float32)
    ba2 = np.asarray(ba2, dtype=np.float32)

    table = node_embeddings.astype(bf16)

    src = action_edges[:, 0].astype(np.int64)
    tgt = action_edges[:, 1].astype(np.int64)

    # ---- per-core bucketing by (src%4, tgt%4)
    cores = []
    for c in range(NC):
        s = src[c * ES:(c + 1) * ES]
        t = tgt[c * ES:(c + 1) * ES]
        key = (s & 3) * 4 + (t & 3)
        order = np.argsort(key, kind="stable")
        counts = np.bincount(key, minlength=16)
        cores.append((s, t, order, counts))

    max_counts = np.max([c[3] for c in cores], axis=0)
    bucket_sizes = [int(-(-m // 128) * 128) for m in max_counts]
    tpad = sum(bucket_sizes)

    nc, tpad2 = _build(bucket_sizes)
    assert tpad2 == tpad

    # ---- combo metadata matrix [120, 65]
    combo_m = np.zeros((128, 4, 65), dtype=np.float32)
    iota = np.arange(MA)
    for k in range(30):
        for bad in range(2):
            for sf in range(2):
                cidx = k * 4 + bad * 2 + sf
                row = np.where(iota <= k - 1, ba2, NEG)
                combo_m[cidx, 3, :MA] = row
                combo_m[cidx, 3, 64] = b2[0] - 1.0 * bad - 100.0 * sf
    # layer-2 weights, paired with lhsT = hts[i] in the device loop:
    # [0] H0 -> edge col, [1] H1 -> edge col, [2] Ha -> army cols
    combo_m[:128, 0, 64] = W2[:128, 0]
    combo_m[:128, 1, 64] = W2[128:, 0]
    combo_m[:, 2, :MA] = Wa2  # [128, 64]

    w1_in = np.zeros((128, 8, 128), dtype=np.float32)
    for k in range(4):
        for m in range(2):
            w1_in[:, k * 2 + m, :] = W1[k * 128:(k + 1) * 128,
                                        m * 128:(m + 1) * 128]
    wa1_in = np.zeros((128, 4, 128), dtype=np.float32)
    for k in range(4):
        wa1_in[:, k, :] = Wa1[k * 128:(k + 1) * 128, :]
    b1_in = np.zeros((128, 3), dtype=np.float32)
    b1_in[:, 0] = b1[:128]
    b1_in[:, 1] = b1[128:]
    b1_in[:, 2] = ba1

    # ---- per-core device inputs
    boffs = np.concatenate([[0], np.cumsum(bucket_sizes)]).astype(np.int64)
    call_layout = []  # (start, n) per call, shared across cores
    for b, pb in enumerate(bucket_sizes):
        off = int(boffs[b])
        for n in _calls_for(pb):
            call_layout.append((off, n))
            off += n

    in_maps = []
    perms = []  # per-core: device position -> original local edge idx (-1 pad)
    for c in range(NC):
        s, t, order, counts = cores[c]
        sl = (s >> 2).astype(np.int16)
        tl = (t >> 2).astype(np.int16)
        sa = army_counts[s].astype(np.int64)
        ta = army_counts[t].astype(np.int64)
        k_ms = np.clip(sa - 1, -1, 28) + 1
        bad = ((sa <= 2) | (ta >= 3 * sa)).astype(np.int64)
        sf = (s == t).astype(np.int64)
        combo = (k_ms * 4 + bad * 2 + sf).astype(np.int64)

        sidx = np.zeros(tpad, dtype=np.int16)
        tidx = np.zeros(tpad, dtype=np.int16)
        ohcol = np.full(tpad, -1, dtype=np.int64)
        perm = np.full(tpad, -1, dtype=np.int64)
        cum = np.concatenate([[0], np.cumsum(counts)]).astype(np.int64)
        for b in range(16):
            ids = order[cum[b]:cum[b + 1]]
            off = int(boffs[b])
            perm[off:off + len(ids)] = ids
            sidx[off:off + len(ids)] = sl[ids]
            tidx[off:off + len(ids)] = tl[ids]
            ohcol[off:off + len(ids)] = combo[ids]

        # wrap indices per gather call: [128, tpad//16]
        sw = np.zeros((128, tpad // 16), dtype=np.int16)
        tw = np.zeros((128, tpad // 16), dtype=np.int16)
        for off, n in call_layout:
            blk_s = sidx[off:off + n].reshape(n // 16, 16).T
            blk_t = tidx[off:off + n].reshape(n // 16, 16).T
            sw[:, off // 16:(off + n) // 16] = np.tile(blk_s, (8, 1))
            tw[:, off // 16:(off + n) // 16] = np.tile(blk_t, (8, 1))

        oh = np.zeros((120, tpad), dtype=bf16)
        valid = ohcol >= 0
        oh[ohcol[valid], np.nonzero(valid)[0]] = bf16(1.0)

        in_maps.append({
            "table": table,
            "sidx": sw,
            "tidx": tw,
            "oh": oh,
            "w1": w1_in.astype(bf16),
            "wa1": wa1_in.astype(bf16),
            "w2x": combo_m.astype(bf16),
            "b1c": b1_in,
        })
        perms.append(perm)

    res = run_bass_kernel_spmd(nc, in_maps, core_ids=list(range(NC)))
    LAST["res"] = res

    edge_logits = np.zeros(N_EDGES, dtype=np.float32)
    army_logits = np.zeros((N_EDGES, MA), dtype=np.float32)
    for c in range(NC):
        out = res.results[c]["out"]  # [128, tpad//128*65]
        dev = out.reshape(128, tpad // 128, 65).transpose(1, 0, 2).reshape(tpad, 65)
        perm = perms[c]
        valid = perm >= 0
        gidx = c * ES + perm[valid]
        edge_logits[gidx] = dev[valid, 64]
        army_logits[gidx] = dev[valid, :MA]
    return edge_logits, army_logits
